# revision 15
# baseline (speedup 1.0000x reference)
"""Elman RNN (CTRNN) Trainium2 kernel — 8-core SPMD.

Problem: h_t = tanh(x_t @ W_in^T + h_{t-1} @ W_hid^T + b), T=512, B=64, I=H=2048.
Outputs: stacked h_1..h_T [T, 1, B, H] and final hx [1, B, H] (== output[-1]).

Strategy (H-sharded recurrence):
  - Each core owns a 256-row slice of the hidden dim (j-range). Weights are
    sliced per core on the host (host also pre-transposes so no on-device
    transposes are needed anywhere).
  - Phase 1: U^T[j_own, t, b] = (x @ W_in^T)^T + bias, computed from a
    pre-transposed x^T (bf16) with W_in^T slices stationary. Entirely
    core-local (j-sliced), no cross-core deps.
  - Phase 2: 512-step scan. Per step, each core computes its h^T chunk
    [256, 64] = tanh(U_t + W_hid^T-slice.T-contraction over full h), with the
    U_t add done on the TensorEngine via an identity-matmul into the same
    PSUM accumulation group. The full h^T [2048, 64] is re-assembled every
    step with one 8-core AllGather over HBM bounce buffers (double-buffered
    by step parity).
  - Output layout per core: [T, 256, 64] fp32; host concatenates/transposes.
"""

import numpy as np
import ml_dtypes

import concourse.bass as bass
import concourse.mybir as mybir
from concourse.bass_utils import run_bass_kernel_spmd

T, B, I, H, L = 512, 64, 2048, 2048, 1
NCORES = 8
JL = H // NCORES          # 256 hidden rows per core
MT = JL // 128            # 2 m-tiles per core
KT = H // 128             # 16 contraction chunks
TB = T * B                # 32768
NB = TB // 512            # 64 phase-1 n-blocks
F32 = mybir.dt.float32
BF16 = mybir.dt.bfloat16
F16 = mybir.dt.float16
AG_GROUPS = [list(range(NCORES))]


def build_nc() -> bass.Bass:
    nc = bass.Bass(num_devices=NCORES, name="ctrnn_scan")

    xT_d = nc.declare_dram_parameter("xT", [I, TB], F16, isOutput=False)
    WiT_d = nc.declare_dram_parameter("WiT", [I, JL], F16, isOutput=False)
    WhT_d = nc.declare_dram_parameter("WhT", [H, JL], F32, isOutput=False)
    bias_d = nc.declare_dram_parameter("bias", [JL, 1], F32, isOutput=False)
    eye_d = nc.declare_dram_parameter("eye", [128, 128], F32, isOutput=False)
    out_d = nc.declare_dram_parameter("out", [T, JL, B], F32, isOutput=True)

    U_d = nc.dram_tensor("U", [MT, 128, T, B], F32)
    cc_in = [nc.dram_tensor(f"cc_in{p}", [128, MT * B], F32) for p in range(2)]
    cc_out = [
        nc.dram_tensor(f"cc_out{p}", [128 * NCORES, MT * B], F32, addr_space="Shared")
        for p in range(2)
    ]

    from contextlib import ExitStack

    with ExitStack() as ctx:
        block = ctx.enter_context(nc.Block())
        s_xt = ctx.enter_context(nc.semaphore("s_xt"))
        s_mm1 = ctx.enter_context(nc.semaphore("s_mm1"))
        s_ub = ctx.enter_context(nc.semaphore("s_ub"))
        s_udma = ctx.enter_context(nc.semaphore("s_udma"))
        s_wload = ctx.enter_context(nc.semaphore("s_wload"))
        s_init = ctx.enter_context(nc.semaphore("s_init"))
        s_u = ctx.enter_context(nc.semaphore("s_u"))
        s_hT = ctx.enter_context(nc.semaphore("s_hT"))
        s_mm = ctx.enter_context(nc.semaphore("s_mm"))
        s_act = ctx.enter_context(nc.semaphore("s_act"))
        s_ccin = ctx.enter_context(nc.semaphore("s_ccin"))
        s_cc = ctx.enter_context(nc.semaphore("s_cc"))
        s_hsdma = ctx.enter_context(nc.semaphore("s_hsdma"))
        sb_WiT = ctx.enter_context(nc.sbuf_tensor("sb_WiT", [128, KT * MT * 128], F16))
        sb_WhT = ctx.enter_context(nc.sbuf_tensor("sb_WhT", [128, KT * MT * 128], F32))
        sb_eye = ctx.enter_context(nc.sbuf_tensor("sb_eye", [128, 128], F32))
        sb_bias = ctx.enter_context(nc.sbuf_tensor("sb_bias", [128, MT], F32))
        sb_xt0 = ctx.enter_context(nc.sbuf_tensor("sb_xt0", [128, KT * 512], F16))
        sb_xt1 = ctx.enter_context(nc.sbuf_tensor("sb_xt1", [128, KT * 512], F16))
        sb_us0 = ctx.enter_context(nc.sbuf_tensor("sb_us0", [128, 512], F32))
        sb_us1 = ctx.enter_context(nc.sbuf_tensor("sb_us1", [128, 512], F32))
        sb_hT0 = ctx.enter_context(nc.sbuf_tensor("sb_hT0", [128, KT * B], F32))
        sb_hT1 = ctx.enter_context(nc.sbuf_tensor("sb_hT1", [128, KT * B], F32))
        sb_u0 = ctx.enter_context(nc.sbuf_tensor("sb_u0", [128, MT * B], F32))
        sb_u1 = ctx.enter_context(nc.sbuf_tensor("sb_u1", [128, MT * B], F32))
        sb_hs0 = ctx.enter_context(nc.sbuf_tensor("sb_hs0", [128, MT * B], F32))
        sb_hs1 = ctx.enter_context(nc.sbuf_tensor("sb_hs1", [128, MT * B], F32))
        ps10 = ctx.enter_context(nc.psum_tensor("ps10", [128, 512], F32))
        ps11 = ctx.enter_context(nc.psum_tensor("ps11", [128, 512], F32))
        ph00 = ctx.enter_context(nc.psum_tensor("ph00", [128, 512], F32))
        ph01 = ctx.enter_context(nc.psum_tensor("ph01", [128, 512], F32))
        ph10 = ctx.enter_context(nc.psum_tensor("ph10", [128, 512], F32))
        ph11 = ctx.enter_context(nc.psum_tensor("ph11", [128, 512], F32))
        sb_xt = [sb_xt0, sb_xt1]
        sb_us = [sb_us0, sb_us1]
        sb_hT = [sb_hT0, sb_hT1]
        sb_u = [sb_u0, sb_u1]
        sb_hs = [sb_hs0, sb_hs1]
        ps1 = [ps10, ps11]
        ps_h = [[ph00, ph01], [ph10, ph11]]  # [parity][m]

        @block.sync
        def _(sp):
            # --- init loads ---
            sp.dma_start(
                out=sb_WiT[:, :].rearrange("p (k q) -> p k q", k=KT),
                in_=WiT_d.ap().rearrange("(k p) q -> p k q", p=128),
            ).then_inc(s_wload, 16)
            sp.dma_start(
                out=sb_WhT[:, :].rearrange("p (k q) -> p k q", k=KT),
                in_=WhT_d.ap().rearrange("(k p) q -> p k q", p=128),
            ).then_inc(s_wload, 16)
            sp.dma_start(out=sb_eye[:, :], in_=eye_d.ap()).then_inc(s_wload, 16)
            with nc.allow_non_contiguous_dma(reason="tiny one-time bias load"):
                sp.dma_start(
                    out=sb_bias[:, :].rearrange("p (m one) -> p m one", one=1),
                    in_=bias_d.ap().rearrange("(m p) one -> p m one", p=128),
                ).then_inc(s_wload, 16)
            # --- phase 1: stream x^T blocks in ---
            for nb in range(NB):
                par = nb % 2
                if nb >= 2:
                    sp.wait_ge(s_mm1, 2 * (nb - 1))  # PE done with block nb-2
                sp.dma_start(
                    out=sb_xt[par][:, :].rearrange("p (k n) -> p k n", k=KT),
                    in_=xT_d.ap()[:, nb * 512 : (nb + 1) * 512].rearrange(
                        "(k p) n -> p k n", p=128
                    ),
                ).then_inc(s_xt, 16)
                for m in range(MT):
                    s = MT * nb + m
                    sp.wait_ge(s_ub, s + 1)  # ustage[s%2] written by ACT
                    t0 = nb * 8
                    sp.dma_start(
                        out=U_d.ap()[m, :, t0 : t0 + 8, :],
                        in_=sb_us[s % 2][:, :],
                    ).then_inc(s_udma, 16)
            # --- phase boundary: all U writes landed ---
            sp.wait_ge(s_udma, 16 * MT * NB)
            # --- scan prologue ---
            sp.dma_start(
                out=sb_u[0][:, :].rearrange("p (m b) -> p m b", m=MT),
                in_=U_d.ap()[:, :, 0, :].rearrange("m p b -> p m b"),
            ).then_inc(s_u, 16)
            # --- scan ---
            for t in range(T):
                par = t % 2
                sp.wait_ge(s_act, 2 * t + 2)  # hstage[par] ready
                if t < T - 1:
                    sp.dma_start(
                        out=cc_in[par].ap(), in_=sb_hs[par][:, :]
                    ).then_inc(s_ccin, 16)
                    # prefetch u[t+1]; WAR: PE(t-1) must have consumed sb_u[(t+1)%2]
                    if t >= 1:
                        sp.wait_ge(s_mm, MT * t)
                    sp.dma_start(
                        out=sb_u[(t + 1) % 2][:, :].rearrange(
                            "p (m b) -> p m b", m=MT
                        ),
                        in_=U_d.ap()[:, :, t + 1, :].rearrange("m p b -> p m b"),
                    ).then_inc(s_u, 16)
                    # gathered h for next step
                    sp.wait_ge(s_cc, t + 1)
                    sp.dma_start(
                        out=sb_hT[(t + 1) % 2][:, :].rearrange(
                            "p (r q) -> p r q", r=NCORES
                        ),
                        in_=cc_out[par].ap().rearrange("(r p) q -> p r q", p=128),
                    ).then_inc(s_hT, 16)
            sp.wait_ge(s_hT, 16 * (T - 1))
            sp.wait_ge(s_ccin, 16 * (T - 1))

        @block.tensor
        def _(pe):
            pe.wait_ge(s_wload, 64)
            # --- phase 1 ---
            for nb in range(NB):
                pe.wait_ge(s_xt, 16 * (nb + 1))
                for m in range(MT):
                    s = MT * nb + m
                    if s >= 2:
                        pe.wait_ge(s_ub, s - 1)  # psum1[s%2] freed by ACT
                    for k in range(KT):
                        mm = pe.matmul(
                            ps1[s % 2][:, 0:512],
                            sb_WiT[:, (k * MT + m) * 128 : (k * MT + m + 1) * 128],
                            sb_xt[nb % 2][:, k * 512 : (k + 1) * 512],
                            start=(k == 0),
                            stop=(k == KT - 1),
                        )
                    mm.then_inc(s_mm1, 1)
            # --- scan ---
            pe.wait_ge(s_init, 1)
            for t in range(T):
                par = t % 2
                pe.wait_ge(s_u, 16 * (t + 1))
                if t >= 1:
                    pe.wait_ge(s_hT, 16 * t)
                if t >= 2:
                    pe.wait_ge(s_act, 2 * (t - 1))  # psum parity freed
                for m in range(MT):
                    pe.matmul(
                        ps_h[par][m][:, 0:B],
                        sb_eye[:, :],
                        sb_u[par][:, m * B : (m + 1) * B],
                        start=True,
                        stop=False,
                    )
                    for k in range(KT):
                        mm = pe.matmul(
                            ps_h[par][m][:, 0:B],
                            sb_WhT[:, (k * MT + m) * 128 : (k * MT + m + 1) * 128],
                            sb_hT[par][:, k * B : (k + 1) * B],
                            start=False,
                            stop=(k == KT - 1),
                        )
                    mm.then_inc(s_mm, 1)

        @block.scalar
        def _(act):
            act.wait_ge(s_wload, 64)
            # --- phase 1: psum -> sbuf staging with bias add ---
            for nb in range(NB):
                for m in range(MT):
                    s = MT * nb + m
                    act.wait_ge(s_mm1, s + 1)
                    if s >= 2:
                        act.wait_ge(s_udma, 16 * (s - 1))  # ustage freed by SP dma
                    act.activation(
                        sb_us[s % 2][:, :],
                        ps1[s % 2][:, 0:512],
                        mybir.ActivationFunctionType.Copy,
                    ).then_inc(s_ub, 1)
            # --- scan: tanh ---
            for t in range(T):
                par = t % 2
                for m in range(MT):
                    act.wait_ge(s_mm, MT * t + m + 1)
                    if t >= 2:
                        act.wait_ge(s_hsdma, 16 * (t - 1))  # out-dma of t-2 done
                        act.wait_ge(s_ccin, 16 * (t - 1))  # cc_in-dma of t-2 done
                    act.activation(
                        sb_hs[par][:, m * B : (m + 1) * B],
                        ps_h[par][m][:, 0:B],
                        mybir.ActivationFunctionType.Tanh,
                        bias=sb_bias[:, m : m + 1],
                    ).then_inc(s_act, 1)

        @block.vector
        def _(vec):
            vec.memset(sb_hT0[:, :], 0.0).then_inc(s_init, 1)

        @block.gpsimd
        def _(g):
            for t in range(T):
                par = t % 2
                if t < T - 1:
                    g.wait_ge(s_ccin, 16 * (t + 1))
                    g.collective_compute(
                        "AllGather",
                        mybir.AluOpType.bypass,
                        replica_groups=AG_GROUPS,
                        ins=[cc_in[par].ap().opt()],
                        outs=[cc_out[par].ap().opt()],
                    ).then_inc(s_cc)
                g.wait_ge(s_act, 2 * t + 2)
                g.dma_start(
                    out=out_d.ap()[t].rearrange("(m p) b -> p m b", p=128),
                    in_=sb_hs[par][:, :].rearrange("p (m b) -> p m b", m=MT),
                ).then_inc(s_hsdma, 16)
            g.wait_ge(s_hsdma, 16 * T)

    return nc


_CACHE: dict = {}


def kernel(input, W_input, W_hidden, bias):
    input = np.asarray(input, dtype=np.float32)
    W_input = np.asarray(W_input, dtype=np.float32)
    W_hidden = np.asarray(W_hidden, dtype=np.float32)
    bias = np.asarray(bias, dtype=np.float32)

    # host-side layout prep (no FLOPs beyond transposes/casts)
    xT = np.ascontiguousarray(input.reshape(TB, I).T).astype(np.float16)
    WiT = np.ascontiguousarray(W_input.T)  # [I, H]
    WhT = np.ascontiguousarray(W_hidden.T)  # [H, H]
    eye = np.eye(128, dtype=np.float32)

    if "nc" not in _CACHE:
        _CACHE["nc"] = build_nc()
    nc = _CACHE["nc"]

    in_maps = []
    for c in range(NCORES):
        j0 = c * JL
        in_maps.append(
            {
                "xT": xT,
                "WiT": np.ascontiguousarray(WiT[:, j0 : j0 + JL]).astype(
                    np.float16
                ),
                "WhT": np.ascontiguousarray(WhT[:, j0 : j0 + JL]),
                "bias": np.ascontiguousarray(bias[j0 : j0 + JL])[:, None],
                "eye": eye,
            }
        )

    res = run_bass_kernel_spmd(nc, in_maps, core_ids=list(range(NCORES)))
    outs = [r["out"] for r in res.results]  # each [T, JL, B]
    full = np.concatenate([o.transpose(0, 2, 1) for o in outs], axis=2)  # [T, B, H]
    output = full[:, None, :, :]  # [T, 1, B, H]
    hx = output[-1]  # [1, B, H]
    return output, hx
